# revision 85
# baseline (speedup 1.0000x reference)
"""Trainium2 Bass kernel for nn_MixedAttnHeadEmbed (mixed-head-config attention).

Math (per batch b):
  Two attention configs share q_m/k_m/v_m [B,T,2048]:
    A: h=8  heads, d_max=256, mixing e in {1024,2048} -> d in {128,256}, weights w0,w1
    B: h=16 heads, d_max=128, mixing e in {1024,2048} -> d in {64,128},  weights w2,w3
  Each config: per-head q/k slices are RoPE'd, weight-summed (padded to d_max),
  GQA (8 kv heads), causal softmax attention; outputs of both configs sum.

Sharding: 8 cores = 4 batches x 2 shards. Shard s owns A-heads [4s,4s+4) and
B-heads [8s,8s+8) -> both write output columns [1024s, 1024s+1024) summed on
device; per-core output is out[t, 1024] (natural row-major orientation).

Device design (cost-model driven):
 - everything bf16 (DVE 2x tensor_tensor, 4x copies; removes the f32r
   small-matmul penalty; halves DMA). Raw q/k/v regions are loaded ONCE and
   sliced per head; per-chunk DMAs are ordered by first consumption.
 - RoPE rotation needs sigma(x) (swap of 64/32-row halves): the host uploads
   sigma-permuted copies of the q/k regions so the rotation costs zero
   on-chip copies; signed sin tables stay in math order.
 - v-mixing (w-weighted sum of the two e-slices) is exactly a linear fold the
   host applies into va2w/vb2w3 during the bf16 cast.
 - scores are computed transposed (sT[k,q]), but y is UNtransposed (y[q,d])
   with pt as the matmul stationary operand: the softmax denominator comes
   from 1-column ones matmuls (~free on the PE: matmul cost is moving-cols
   only) and lands on q-partitions, so normalization is a per-partition
   broadcast multiply.
 - causal diag mask added on the PE (identity-stationary matmul of a mask
   tile) instead of a DVE pass.
 - per (head, chunk) the score psum is one [P,1024] 2-bank tile -> ONE exp
   instruction over [128c, T); max-free softmax (scores are provably small
   for this problem family; exp is safe in fp32).
 - PSUM accumulation groups share banks; exactly one start=True matmul per
   bank (emitted first) pre-zeroes the bank for all groups in it.
 - elementwise ops are load-balanced across DVE/Pool/ACT by a static
   cost-model-aware picker.
"""

import math
from contextlib import ExitStack
from dataclasses import dataclass

import numpy as np

import concourse.bass as bass
import concourse.mybir as mybir
import concourse.tile as tile
from concourse import bacc

F32 = mybir.dt.float32
BF16 = mybir.dt.bfloat16
NEG = -1e9
MASKNEG = -30000.0
P = 128


@dataclass(frozen=True)
class KCfg:
    T: int = 1024       # sequence length
    NA: int = 4         # config-A heads per core (d_max=256)
    NB: int = 8         # config-B heads per core (d_max=128)

    @property
    def TK(self):
        return self.T // P


FULL = KCfg()


def _in_specs(cfg: KCfg):
    T = cfg.T
    return {
        "qT1": (cfg.NA * 128, T),    # q d=128 slices, transposed
        "qT2": (cfg.NA * 256, T),    # q d=256 slices (also B d=128 slices)
        "kTa1": (cfg.NA * 128, T),   # k d=128 slices (A and B share)
        "kTa2": (cfg.NA * 256, T),   # k d=256 slices
        "kTb1": (cfg.NA * 64, T),    # k d=64 slices (B)
        "qT1s32": (cfg.NA * 128, T),  # sigma32-permuted qT1 (B d64 rope)
        "qT1s64": (cfg.NA * 128, T),  # sigma64-permuted qT1 (A d128 rope)
        "kTa1s64": (cfg.NA * 128, T),  # sigma64 kTa1 (A + B-k d128 rope)
        "kTb1s32": (cfg.NA * 64, T),   # sigma32 kTb1 (B d64 rope)
        "qT2s64": (cfg.NA * 256, T),   # sigma64 qT2 (B-q d128 rope)
        "vb2w3": (T, cfg.NA * 128),  # B v-mix, fully host-folded (w3*v2+w2*v1pad)
        "va2w": (T, cfg.NA * 256),   # A v-mix, host-folded (w1*v2 + w0*v1 in dc0-lo)
        "tabA2": (256, T),   # [ca2; sa2] rope-256 tables (fill-critical)
        "tabA1": (256, T),   # [ca1; sa1]
        "tabB": (512, T),    # [cb2; sb2; cb1; sb1]
    }


class _EngPick:
    """Cost-aware static load balancer.

    ns costs per 1024-col op (TRN2 v1 cost model, bf16 sbuf operands):
      tensor_tensor: DVE 594 (2x mode) / Pool 853
      copy:          DVE 327 (4x mode) / Pool 850 / ACT 1038
      stt/ts (sbuf): DVE 1127 / Pool 853
    ACT additionally carries all exps; PSUM-touching ops are DVE-only."""

    def __init__(self, nc):
        self.nc = nc
        self.load = {"dve": 0.0, "pool": 0.0, "act": 0.0}

    def _pick(self, costs):
        eng = min(costs, key=lambda k: self.load[k] + costs[k])
        self.load[eng] += costs[eng]
        return eng

    def tt(self, cols=1024):
        f = cols / 1024.0
        eng = self._pick({"dve": 594 * f, "pool": 853 * f})
        return self.nc.vector if eng == "dve" else self.nc.gpsimd

    def stt(self, cols=1024):
        # TensorScalarPtr only exists on DVE (Pool rejects it in codegen)
        self.load["dve"] += 1127 * cols / 1024.0
        return self.nc.vector

    def copy(self, dst, src, cols=1024):
        f = cols / 1024.0
        eng = self._pick({"dve": 327 * f, "pool": 850 * f, "act": 1038 * f})
        if eng == "act":
            self.nc.scalar.copy(dst, src)
        elif eng == "pool":
            self.nc.gpsimd.tensor_copy(dst, src)
        else:
            self.nc.vector.tensor_copy(dst, src)

    def dve(self, ns):
        self.load["dve"] += ns
        return self.nc.vector

    def act(self, ns):
        self.load["act"] += ns
        return self.nc.scalar


def build_program(cfg: KCfg = FULL):
    nc = bacc.Bacc("TRN2", target_bir_lowering=False,
                   dynamic_dma_scratch_size=1024)
    T, TK = cfg.T, cfg.TK
    mult, add = mybir.AluOpType.mult, mybir.AluOpType.add
    EXP = mybir.ActivationFunctionType.Exp

    D = {}
    for name, shape in _in_specs(cfg).items():
        D[name] = nc.declare_dram_parameter(name, list(shape), BF16, isOutput=False)
    outD = nc.declare_dram_parameter("out", [T, 1024], BF16, isOutput=True)

    with ExitStack() as ctx:
        tc = ctx.enter_context(tile.TileContext(nc))
        const = ctx.enter_context(tc.tile_pool(name="const", bufs=1))
        raw = ctx.enter_context(tc.tile_pool(name="raw", bufs=1))
        mixp = ctx.enter_context(tc.tile_pool(name="mix", bufs=2))
        scr = ctx.enter_context(tc.tile_pool(name="scr", bufs=2))
        ptp = ctx.enter_context(tc.tile_pool(name="pt", bufs=8))
        recp = ctx.enter_context(tc.tile_pool(name="rec", bufs=2))
        accp = ctx.enter_context(tc.tile_pool(name="acc", bufs=1))
        spsum = ctx.enter_context(tc.tile_pool(name="spsum", bufs=2, space="PSUM"))
        ypsum = ctx.enter_context(tc.tile_pool(name="ypsum", bufs=2, space="PSUM"))
        dpsum = ctx.enter_context(tc.tile_pool(name="dpsum", bufs=2, space="PSUM"))

        pick = _EngPick(nc)

        # ---- constants ----
        ident = const.tile([P, P], BF16, name="ident")
        nc.gpsimd.memset(ident, 1.0)
        # keep where q - p >= 0, else 0 ; then keep where q - p <= 0 -> diag
        nc.gpsimd.affine_select(out=ident, in_=ident,
                                compare_op=mybir.AluOpType.is_ge, fill=0.0,
                                base=0, pattern=[[1, P]], channel_multiplier=-1)
        nc.gpsimd.affine_select(out=ident, in_=ident,
                                compare_op=mybir.AluOpType.is_ge, fill=0.0,
                                base=0, pattern=[[-1, P]], channel_multiplier=1)
        maskM = const.tile([P, P], BF16, name="maskM")
        nc.gpsimd.memset(maskM, 0.0)
        # maskM[k, q] = 0 where q >= k else MASKNEG (transposed causal diag blk)
        nc.gpsimd.affine_select(out=maskM, in_=maskM,
                                compare_op=mybir.AluOpType.is_ge, fill=MASKNEG,
                                base=0, pattern=[[1, P]], channel_multiplier=-1)
        onescol = const.tile([P, 1], BF16, name="onescol")
        nc.vector.memset(onescol, 1.0)

        # ---- tables + raw inputs, DMA'd in consumption order ----
        # tables first (every mix needs them), then per-head chunk DMAs so
        # head 0's mixing can start ~5us in instead of after all input DMAs.
        tabs = {}

        def load_tab(nm, parts):
            rows = _in_specs(cfg)[nm][0]
            tl = const.tile([P, rows // P, T], BF16, name=nm, tag=nm)
            nc.sync.dma_start(out=tl, in_=D[nm].rearrange("(c p) t -> p c t", p=P))
            for i, p_ in enumerate(parts):
                tabs[p_] = tl[:, i:i + 1, :]

        # only ca2/sa2 up front: the first mix ops (A-q dc1) need just these
        # plus qT2 chunk 0; the rest loads interleaved below.
        load_tab("tabA2", ("ca2", "sa2"))

        R = {}
        for nm in ("qT1", "kTa1", "kTb1", "qT2", "kTa2",
                   "qT1s32", "qT1s64", "kTa1s64", "kTb1s32", "qT2s64"):
            rows = _in_specs(cfg)[nm][0]
            R[nm] = raw.tile([P, rows // P, T], BF16, name=nm, tag=nm)
        for nm in ("vb2w3", "va2w"):
            cols = _in_specs(cfg)[nm][1]
            R[nm] = raw.tile([P, TK, cols], BF16, name=nm, tag=nm)

        def dma_rows(nm, c0, c1):
            nc.sync.dma_start(
                out=R[nm][:, c0:c1, :],
                in_=D[nm].rearrange("(c p) t -> p c t", p=P)[:, c0:c1, :])

        def dma_vcols(nm, d0, d1):
            nc.sync.dma_start(
                out=R[nm][:, :, d0:d1],
                in_=D[nm].rearrange("(c p) d -> p c d", p=P)[:, :, d0:d1])

        for h in range(cfg.NA):
            dma_rows("qT2", 2 * h, 2 * h + 2)
            if h == 0:
                load_tab("tabA1", ("ca1", "sa1"))
            dma_rows("qT1", h, h + 1)
            dma_rows("qT1s64", h, h + 1)
            dma_rows("kTa2", 2 * h, 2 * h + 2)
            dma_rows("kTa1", h, h + 1)
            dma_rows("kTa1s64", h, h + 1)
            if h == 0:
                # B tables only needed once the trio-0 B mixes start
                load_tab("tabB", ("cb2", "sb2", "cb1", "sb1"))
            dma_rows("qT2s64", 2 * h, 2 * h + 2)
            dma_rows("qT1s32", h, h + 1)
            # whole-tensor v loads (row-contiguous, no small-elem penalty)
            if h == 0:
                dma_rows("kTb1", 0, 1)
                dma_rows("kTb1s32", 0, 1)
                dma_vcols("va2w", 0, 512)
                dma_vcols("vb2w3", 0, 512)
            elif h == 2:
                dma_vcols("va2w", 512, 1024)
                dma_rows("kTb1", 1, 2)
                dma_rows("kTb1s32", 1, 2)

        outacc = accp.tile([P, TK, 1024], BF16)

        def sig64(dst, u):
            """dst = swap 64-halves of u (cross-partition-base copies)."""
            pick.copy(dst[0:64, :], u[64:128, :])
            pick.copy(dst[64:128, :], u[0:64, :])

        def sig32(dst, u, base=0, rows=P):
            for g in range(rows // 64):
                b0 = base + 64 * g
                pick.copy(dst[b0:b0 + 32, :], u[b0 + 32:b0 + 64, :])
                pick.copy(dst[b0 + 32:b0 + 64, :], u[b0:b0 + 32, :])

        def mix_A(x1, x1s, x2, tag):
            """[P,2,T] bf16 mix for one config-A head side.
            x1 [P,T] raw d=128 slice; x1s its sigma64-permuted copy (host
            uploads the permuted rows, so no on-chip rotation copies);
            x2 [P,2,T] raw d=256 slice."""
            out = mixp.tile([P, 2, T], BF16, tag=tag)
            t1 = scr.tile([P, T], BF16, tag="t1", bufs=2)
            t2 = scr.tile([P, T], BF16, tag="t2", bufs=2)
            ca1, sa1 = tabs["ca1"], tabs["sa1"]
            ca2, sa2 = tabs["ca2"], tabs["sa2"]  # [P,1,T]; rope-256 halves repeat
            # dc1 = x2_1*c2 + x2_0*s2
            pick.tt().tensor_tensor(out[:, 1, :], x2[:, 1, :], ca2[:, 0, :], mult)
            pick.tt().tensor_tensor(t1, x2[:, 0, :], sa2[:, 0, :], mult)
            pick.tt().tensor_tensor(out[:, 1, :], out[:, 1, :], t1, add)
            # dc0 = (x2_0*c2 - x2_1*s2) + (x1*c1 + sig64(x1)*s1), as a
            # balanced tree: 4 independent mults, 2 parallel combines, 1 add
            t3 = scr.tile([P, T], BF16, tag="t3")
            pick.tt().tensor_tensor(out[:, 0, :], x2[:, 0, :], ca2[:, 0, :], mult)
            pick.tt().tensor_tensor(t1, x2[:, 1, :], sa2[:, 0, :], mult)
            pick.tt().tensor_tensor(t2, x1, ca1[:, 0, :], mult)
            pick.tt().tensor_tensor(t3, x1s, sa1[:, 0, :], mult)
            pick.tt().tensor_tensor(out[:, 0, :], out[:, 0, :], t1,
                                    mybir.AluOpType.subtract)
            pick.tt().tensor_tensor(t2, t2, t3, add)
            pick.tt().tensor_tensor(out[:, 0, :], out[:, 0, :], t2, add)
            return out

        def mix_B128(x2, x2s, ctab, stab, tag, bufs=None):
            """[P,T] bf16 rope-128: x2*c + sig64(x2)*s (x2s host-permuted)."""
            out = mixp.tile([P, T], BF16, tag=tag, bufs=bufs)
            t2 = scr.tile([P, T], BF16, tag="t2", bufs=2)
            pick.tt().tensor_tensor(out, x2, ctab[:, 0, :], mult)
            pick.tt().tensor_tensor(t2, x2s, stab[:, 0, :], mult)
            pick.tt().tensor_tensor(out, out, t2, add)
            return out

        def mix_B64pair(x1p, x1ps, tag, bufs=None):
            """[P,T] rope-64 of a packed pair (two 64-row d=64 slices)."""
            out = mixp.tile([P, T], BF16, tag=tag, bufs=bufs)
            t2 = scr.tile([P, T], BF16, tag="t2", bufs=2)
            cb1, sb1 = tabs["cb1"], tabs["sb1"]
            pick.tt().tensor_tensor(out, x1p, cb1[:, 0, :], mult)
            pick.tt().tensor_tensor(t2, x1ps, sb1[:, 0, :], mult)
            pick.tt().tensor_tensor(out, out, t2, add)
            return out

        def attn_head(qmixs, kmixs, vget, dwid, out_lo, is_b):
            """One attention head, untransposed-y layout.

            qmixs/kmixs: list of [P, T] APs per 128-d-chunk.
            vget: fn c -> [P, dwid] moving-V AP for that k-chunk.
            dwid: output width (256 A / 128 B); out_lo: outacc col offset.

            B heads (dwid=128): y runs inline in the c-loop with both
            [P,4,128] qb-half psum tiles live (pt tiles free immediately).
            A heads (dwid=256): two [P,4,256] y passes over the saved pts.
            """
            ndc = len(qmixs)
            den = dpsum.tile([P, 512], F32, tag="den", name="den")
            rec = recp.tile([P, 8], F32, tag="rec", name="rec")
            pts = []

            def norm(ypt, qb0, nq, lo, wid):
                # normalize: rec broadcast along out cols (stride-0 AP)
                rb = rec[:, qb0:qb0 + nq].unsqueeze(2) \
                    .broadcast_to([P, nq, wid])
                osl = outacc[:, qb0:qb0 + nq, lo:lo + wid]
                f = wid * nq / 1024.0
                if not is_b:
                    pick.dve(133 + 1067 * f).tensor_tensor(osl, ypt, rb, mult)
                else:
                    tmp = scr.tile([P, nq, wid], BF16, tag="ntmp", name="ntmp")
                    pick.dve(133 + 1067 * f).tensor_tensor(tmp, ypt, rb, mult)
                    nc.gpsimd.tensor_tensor(osl, osl, tmp, add)

            def pv(c, qb):
                tile_, delta = pts[c]
                return tile_[:, P * qb - delta:P * qb - delta + P]

            def emit_groups(groups, cmax):
                # y matmuls over saved pts for the given qb groups (all of
                # whose den columns are final by chunk cmax), then normalize
                for (qb0, nq, wid) in groups:
                    yp = ypsum.tile([P, nq, wid], F32, tag="yp", name="ypg")
                    qbs = list(range(qb0 + nq - 1, qb0 - 1, -1))
                    for c in range(cmax + 1):
                        for qb in qbs:
                            if qb < c:
                                continue
                            nc.tensor.matmul(
                                yp[:, qb - qb0, :],
                                pv(c, qb), vget(c),
                                start=(c == 0 and qb == qbs[0]),
                                stop=(c == qb), skip_group_check=True)
                    norm(yp, qb0, nq, out_lo, wid)

            def dens(c):
                for qb in range(TK - 1, c - 1, -1):
                    nc.tensor.matmul(
                        den[:, qb:qb + 1], pv(c, qb), onescol,
                        start=(c == 0 and qb == TK - 1),
                        stop=(c == qb), skip_group_check=True)

            def score_group(sT, a, q0, kq_hi, is_first_in_bank):
                """Score matmuls for chunk q0//P into sT cols [a, a+n);
                the diag block sits at [a, a+P)."""
                n = kq_hi - q0
                for dc in range(ndc):
                    nc.tensor.matmul(
                        sT[:, a:a + n],
                        kmixs[dc][:, q0:q0 + P],
                        qmixs[dc][:, q0:kq_hi],
                        start=(dc == 0 and is_first_in_bank),
                        stop=False, skip_group_check=True)
                nc.tensor.matmul(
                    sT[:, a:a + P], ident, maskM,
                    start=False, stop=True, skip_group_check=True)

            for c in range(5):
                q0 = P * c
                # one [P, T] f32 sT tile (2 banks); bank-aligned score groups,
                # then ONE exp instruction over the contiguous [q0, T) range
                sT = spsum.tile([P, T], F32, tag="sT", name="sT")
                pt = ptp.tile([P, T], BF16, tag="pt", name="pt")
                pts.append((pt, 0))
                if c < 4:
                    # off-diag upper seg is its own bank-1 group
                    for dc in range(ndc):
                        nc.tensor.matmul(
                            sT[:, 512:1024],
                            kmixs[dc][:, q0:q0 + P],
                            qmixs[dc][:, 512:1024],
                            start=(dc == 0), stop=(dc == ndc - 1),
                            skip_group_check=True)
                    score_group(sT, q0, q0, 512, True)
                else:
                    score_group(sT, q0, q0, 1024, True)
                pick.act((T - q0) * 0.833 + 185).activation(
                    pt[:, q0:T], sT[:, q0:T], EXP)
                dens(c)
            # chunks 5..7 (384+128+256 cols) pack into ONE sT tile and ONE
            # exp: c5 -> [0:384) bank0, c7 -> [384:512) bank0, c6 -> [512:768)
            sTp = spsum.tile([P, T], F32, tag="sT", name="sTp")
            ptpk = ptp.tile([P, T], BF16, tag="pt", name="ptpk")
            pts.append((ptpk, 640))   # c5: local 0   = q 640
            pts.append((ptpk, 256))   # c6: local 512 = q 768
            pts.append((ptpk, 512))   # c7: local 384 = q 896
            score_group(sTp, 0, 640, 1024, True)
            score_group(sTp, 512, 768, 1024, True)
            score_group(sTp, 384, 896, 1024, False)
            pick.act(768 * 0.833 + 185).activation(
                ptpk[:, 0:768], sTp[:, 0:768], EXP)
            for c in (5, 6, 7):
                dens(c)

            pick.dve(140).reciprocal(rec, den[:, 0:8])
            emit_groups([(6, 2, 256), (4, 2, 256), (2, 2, 256), (0, 2, 256)]
                        if not is_b else [(4, 4, P), (0, 4, P)], TK - 1)


        def do_A(h):
            qmix = mix_A(R["qT1"][:, h, :], R["qT1s64"][:, h, :],
                         R["qT2"][:, 2 * h:2 * h + 2, :], "qmixA")
            kmix = mix_A(R["kTa1"][:, h, :], R["kTa1s64"][:, h, :],
                         R["kTa2"][:, 2 * h:2 * h + 2, :], "kmixA")
            return qmix, kmix

        def attn_A(h, am):
            qmix, kmix = am
            va2 = R["va2w"]
            attn_head([qmix[:, 0, :], qmix[:, 1, :]],
                      [kmix[:, 0, :], kmix[:, 1, :]],
                      lambda c: va2[:, c, 256 * h:256 * h + 256],
                      256, 256 * h, is_b=False)

        # B kv-head state, computed per kv j (shared by B-heads 2j, 2j+1)
        bkv = {}

        def prep_Bkv(j):
            kmix = mix_B128(R["kTa1"][:, j, :], R["kTa1s64"][:, j, :],
                            tabs["cb2"], tabs["sb2"], "kmixB", bufs=3)
            u = j // 2
            kd64 = bkv.get(("kd64", u))
            if kd64 is None:
                kd64 = mix_B64pair(R["kTb1"][:, u, :], R["kTb1s32"][:, u, :],
                                   "kd64B", bufs=1)
                bkv[("kd64", u)] = kd64
            half = 0 if j % 2 == 0 else 64
            if half == 0:
                pick.tt().tensor_tensor(kmix[0:64, :], kmix[0:64, :],
                                        kd64[0:64, :], add)
            else:
                t2 = scr.tile([P, T], BF16, tag="t2", name="xb", bufs=2)
                pick.copy(t2[0:64, :], kd64[64:128, :])
                pick.tt().tensor_tensor(kmix[0:64, :], kmix[0:64, :],
                                        t2[0:64, :], add)
            bkv[("kmix", j)] = kmix

        def mix_Bq_pair(u):
            """d128 rope for the B-head pair (2u, 2u+1) in merged 2048-col
            ops (tables broadcast across the pair with a stride-0 AP)."""
            x2 = R["qT2"][:, 2 * u:2 * u + 2, :]
            x2s = R["qT2s64"][:, 2 * u:2 * u + 2, :]
            cb = tabs["cb2"][:, 0, :].unsqueeze(1).broadcast_to([P, 2, T])
            sb = tabs["sb2"][:, 0, :].unsqueeze(1).broadcast_to([P, 2, T])
            qp = mixp.tile([P, 2, T], BF16, tag="qmixBp", bufs=2)
            t2p = scr.tile([P, 2, T], BF16, tag="t2p", name="t2p", bufs=1)
            pick.tt(2048).tensor_tensor(qp, x2, cb, mult)
            pick.tt(2048).tensor_tensor(t2p, x2s, sb, mult)
            pick.tt(2048).tensor_tensor(qp, qp, t2p, add)
            # fold the packed d64 pair into rows 0:64 of each head
            qd64 = mix_B64pair(R["qT1"][:, u, :], R["qT1s32"][:, u, :],
                               "qd64B", bufs=1)
            pick.tt().tensor_tensor(qp[0:64, 0, :], qp[0:64, 0, :],
                                    qd64[0:64, :], add)
            t2 = scr.tile([P, T], BF16, tag="t2", name="xb2", bufs=2)
            pick.copy(t2[0:64, :], qd64[64:128, :])
            pick.tt().tensor_tensor(qp[0:64, 1, :], qp[0:64, 1, :],
                                    t2[0:64, :], add)
            return qp

        def attn_B(hh, qmix):
            j = hh // 2
            vb = R["vb2w3"]
            attn_head([qmix], [bkv[("kmix", j)]],
                      lambda c: vb[:, c, P * j:P * j + P],
                      128, 128 * hh, is_b=True)

        # per trio (A_h, B_2h, B_2h+1): emit all mixes first so DVE/Pool
        # front-run the next trio while PE/ACT drain the previous one
        with nc.allow_low_precision(reason="bf16 attention"):
            for h in range(cfg.NA):
                am = do_A(h)
                prep_Bkv(h)
                qp = mix_Bq_pair(h)
                attn_A(h, am)
                attn_B(2 * h, qp[:, 0, :])
                attn_B(2 * h + 1, qp[:, 1, :])
                # output block [*, 256h:256h+256] is final; split by
                # q-half so the first half overlaps the second half's norms
                for (c0, c1) in ((4, 8), (0, 4)):
                    nc.sync.dma_start(
                        out=outD[:, 256 * h:256 * h + 256]
                        .rearrange("(c p) d -> p c d", p=P)[:, c0:c1, :],
                        in_=outacc[:, c0:c1, 256 * h:256 * h + 256])

    nc.compile()
    return nc


# ---------------------------------------------------------------------------
# Host side
# ---------------------------------------------------------------------------

def _rope_tab(pos, d, f):
    """Transposed rope tables [d, T]: (f*cos, +-f*sin with rot sign folded)."""
    inv = 1.0 / (10000.0 ** (np.arange(0, d, 2, dtype=np.float32) / d))
    ang = inv[:, None] * pos[None, :].astype(np.float32)      # [d/2, T]
    ang = np.concatenate([ang, ang], 0)                        # [d, T]
    c = (f * np.cos(ang)).astype(np.float32)
    s = (f * np.sin(ang)).astype(np.float32)
    s[: d // 2] *= -1.0
    return c, s


def _fold_va(v, w, s):
    """A v-mix, host-folded: w1*v_256slices with w0*v_128slices added into
    the dc0-lo half of each head block."""
    import ml_dtypes
    out = w[1] * v[:, 1024 * s:1024 * s + 1024]
    for h in range(4):
        out[:, 256 * h:256 * h + 128] += \
            w[0] * v[:, 512 * s + 128 * h:512 * s + 128 * h + 128]
    return np.ascontiguousarray(out).astype(ml_dtypes.bfloat16)


def _fold_vb(v, w, s):
    """B v-mix, host-folded: w3*v_128slices with w2*v_64slices added into
    the lo half of each kv block."""
    import ml_dtypes
    out = w[3] * v[:, 512 * s:512 * s + 512]
    for j in range(4):
        out[:, 128 * j:128 * j + 64] += \
            w[2] * v[:, 256 * s + 64 * j:256 * s + 64 * j + 64]
    return np.ascontiguousarray(out).astype(ml_dtypes.bfloat16)


def make_core_inputs(q, k, v, pos, weights, s, cfg: KCfg = FULL):
    """q,k,v: [T, 2048] f32 for one batch; returns per-core input dict."""
    import ml_dtypes
    bf = ml_dtypes.bfloat16
    c = np.ascontiguousarray
    w = np.asarray(weights, np.float32)
    def sigrows(t, half):
        # swap `half`-row blocks within each 2*half group (rope rotation)
        return np.ascontiguousarray(
            t.reshape(-1, 2, half, t.shape[-1])[:, ::-1].reshape(t.shape))

    qT1 = c(q[:, 512 * s:512 * s + 512].T)
    qT2 = c(q[:, 1024 * s:1024 * s + 1024].T)
    kTa1 = c(k[:, 512 * s:512 * s + 512].T)
    kTb1 = c(k[:, 256 * s:256 * s + 256].T)
    arrs = {
        "qT1": qT1.astype(bf),
        "qT2": qT2.astype(bf),
        "kTa1": kTa1.astype(bf),
        "kTa2": c(k[:, 1024 * s:1024 * s + 1024].T).astype(bf),
        "kTb1": kTb1.astype(bf),
        "qT1s32": sigrows(qT1, 32).astype(bf),
        "qT1s64": sigrows(qT1, 64).astype(bf),
        "kTa1s64": sigrows(kTa1, 64).astype(bf),
        "kTb1s32": sigrows(kTb1, 32).astype(bf),
        "qT2s64": sigrows(qT2, 64).astype(bf),
        "vb2w3": _fold_vb(v, w, s),
        "va2w": _fold_va(v, w, s),
    }
    fA = math.sqrt(1.0 / 16.0)
    fB = math.sqrt(1.0 / math.sqrt(128.0))
    ca1, sa1 = _rope_tab(pos, 128, fA * float(w[0]))
    ca2, sa2 = _rope_tab(pos, 256, fA * float(w[1]))
    cb1h, sb1h = _rope_tab(pos, 64, fB * float(w[2]))
    cb2, sb2 = _rope_tab(pos, 128, fB * float(w[3]))

    def sigma(tab, half):
        out = tab.reshape(-1, 2, half, tab.shape[-1])
        return np.ascontiguousarray(out[:, ::-1].reshape(tab.shape))

    arrs.update({
        # math-order signed-sin tables (data side is pre-permuted), packed
        # into combined tensors so each group is one DMA
        "tabA2": np.vstack([ca2[:128], sa2[128:]]).astype(bf),
        "tabA1": np.vstack([ca1, sa1]).astype(bf),
        "tabB": np.vstack([cb2, sb2, cb1h, cb1h, sb1h, sb1h]).astype(bf),
    })
    return arrs


_PROGRAM_CACHE = {}
TRACE = False
LAST_RESULT = None


def kernel(q_m, k_m, v_m, weights, attention_mask, position_ids):
    global LAST_RESULT
    from concourse.bass_utils import run_bass_kernel_spmd

    cfg = FULL
    q_m = np.asarray(q_m, np.float32)
    k_m = np.asarray(k_m, np.float32)
    v_m = np.asarray(v_m, np.float32)
    weights = np.asarray(weights, np.float32)
    attention_mask = np.asarray(attention_mask, np.float32)
    position_ids = np.asarray(position_ids)
    B, T, H = q_m.shape

    causal = np.where(np.tril(np.ones((T, T), bool)), 0.0, NEG).astype(np.float32)
    for b in range(B):
        assert np.array_equal(attention_mask[b, 0], causal), "non-causal mask"

    if "nc" not in _PROGRAM_CACHE:
        _PROGRAM_CACHE["nc"] = build_program(cfg)
    nc = _PROGRAM_CACHE["nc"]

    in_maps = []
    for b in range(B):
        for s in range(2):
            in_maps.append(make_core_inputs(
                q_m[b], k_m[b], v_m[b], position_ids[b], weights, s, cfg))
    res = run_bass_kernel_spmd(nc, in_maps, list(range(8)), trace=TRACE)
    LAST_RESULT = res
    out = np.zeros((B, T, H), np.float32)
    for b in range(B):
        for s in range(2):
            out[b, :, 1024 * s:1024 * s + 1024] = \
                res.results[2 * b + s]["out"].astype(np.float32)
    return out
